# revision 4
# baseline (speedup 1.0000x reference)
"""Trainium2 Bass kernel for an 8-expert top-2 MoE layer.

Strategy (expert-parallel, per the sharding hint "all-to-all tokens by
top-k assignment"): the host computes the (tiny) gating matmul + softmax
+ top-2 routing, gathers each expert's assigned tokens, and ships one
expert per NeuronCore. Each core runs the heavy 2-layer MLP for its
expert over its assigned tokens (f32r matmuls on the PE array), applies
the gate weights on-device, and the host scatter-adds the two expert
contributions per token back together.

The MLP math runs fully transposed (tokens on the free dim) so that
 - W1/W2 slices feed the PE as stationary operands with no transposes,
 - the b1 bias + relu and (y + b2) * gate evictions are single fused
   DVE ops with per-partition scalars,
 - the per-token gate row is broadcast across partitions with one K=1
   matmul (ones[1,128]^T @ g[1,N] -> G[128,N]).
"""

import numpy as np

NUM_EXPERTS = 8
TOP_K = 2
D = 1024
TOK_TILE = 512

_prog_cache = {}


def _build_program(C):
    """Build the per-core Bass program: one expert's MLP over C tokens."""
    from contextlib import ExitStack

    import concourse.tile as tile
    from concourse import bacc, mybir

    f32 = mybir.dt.float32
    f32r = mybir.dt.float32r
    ADD = mybir.AluOpType.add
    MAX = mybir.AluOpType.max
    MULT = mybir.AluOpType.mult

    assert C % TOK_TILE == 0
    NT = C // TOK_TILE

    nc = bacc.Bacc("TRN2", target_bir_lowering=False, debug=False,
                   num_devices=NUM_EXPERTS)

    xT_d = nc.dram_tensor("xT", [D, C], f32r, kind="ExternalInput").ap()
    w1_d = nc.dram_tensor("w1", [D, D], f32r, kind="ExternalInput").ap()
    w2_d = nc.dram_tensor("w2", [D, D], f32r, kind="ExternalInput").ap()
    b1_d = nc.dram_tensor("b1", [8, 128, 1], f32, kind="ExternalInput").ap()
    b2_d = nc.dram_tensor("b2", [8, 128, 1], f32, kind="ExternalInput").ap()
    g_d = nc.dram_tensor("g", [1, C], f32r, kind="ExternalInput").ap()
    ones_d = nc.dram_tensor("ones", [1, 128], f32r, kind="ExternalInput").ap()
    yT_d = nc.dram_tensor("yT", [D, C], f32, kind="ExternalOutput").ap()

    with tile.TileContext(nc) as tc, ExitStack() as ctx:
        wpool = ctx.enter_context(tc.tile_pool(name="w", bufs=1))
        cpool = ctx.enter_context(tc.tile_pool(name="const", bufs=1))
        xpool = ctx.enter_context(tc.tile_pool(name="x", bufs=2))
        hpool = ctx.enter_context(tc.tile_pool(name="h", bufs=2))
        ypool = ctx.enter_context(tc.tile_pool(name="y", bufs=2))
        gpool = ctx.enter_context(tc.tile_pool(name="g", bufs=2))
        php = ctx.enter_context(tc.tile_pool(name="ph", bufs=3, space="PSUM"))
        pyp = ctx.enter_context(tc.tile_pool(name="py", bufs=3, space="PSUM"))
        pgp = ctx.enter_context(tc.tile_pool(name="pg", bufs=2, space="PSUM"))

        # resident weights: 8 partition-tiles each of [128, 1024]
        w1_sb = []
        w2_sb = []
        for i in range(8):
            t1 = wpool.tile([128, D], f32r, tag=f"w1_{i}")
            nc.sync.dma_start(t1[:], w1_d[i * 128:(i + 1) * 128, :])
            w1_sb.append(t1)
            t2 = wpool.tile([128, D], f32r, tag=f"w2_{i}")
            nc.sync.dma_start(t2[:], w2_d[i * 128:(i + 1) * 128, :])
            w2_sb.append(t2)

        # biases as per-partition columns: b1_sb[:, j] = b1[j*128 : (j+1)*128]
        b1_sb = cpool.tile([128, 8], f32, tag="b1")
        b2_sb = cpool.tile([128, 8], f32, tag="b2")
        for j in range(8):
            nc.sync.dma_start(b1_sb[:, j:j + 1], b1_d[j])
            nc.sync.dma_start(b2_sb[:, j:j + 1], b2_d[j])

        # gate row + ones row (for the partition-broadcast matmul)
        g_sb = cpool.tile([1, C], f32r, tag="g")
        nc.sync.dma_start(g_sb[:], g_d[:])
        ones_sb = cpool.tile([1, 128], f32r, tag="ones")
        nc.sync.dma_start(ones_sb[:], ones_d[:])

        for t in range(NT):
            tsl = slice(t * TOK_TILE, (t + 1) * TOK_TILE)

            # token tile of x^T: 8 partition-tiles [128, TOK_TILE]
            x_sb = []
            for d in range(8):
                xt = xpool.tile([128, TOK_TILE], f32r, tag=f"x{d}")
                nc.sync.dma_start(xt[:], xT_d[d * 128:(d + 1) * 128, tsl])
                x_sb.append(xt)

            # broadcast gate row across partitions: G[p, n] = g[n]
            g_ps = pgp.tile([128, TOK_TILE], f32, tag="gps")
            nc.tensor.matmul(g_ps[:], ones_sb[:], g_sb[:, tsl],
                             start=True, stop=True)
            g_bc = gpool.tile([128, TOK_TILE], f32, tag="gbc")
            nc.vector.tensor_copy(g_bc[:], g_ps[:])

            # layer 1: h^T[j,:] = relu(sum_d W1[d,j]^T x^T[d,:] + b1[j])
            h_sb = []
            for j in range(8):
                ph = php.tile([128, TOK_TILE], f32, tag="ph")
                for d in range(8):
                    nc.tensor.matmul(ph[:],
                                     w1_sb[d][:, j * 128:(j + 1) * 128],
                                     x_sb[d][:],
                                     start=(d == 0), stop=(d == 7))
                ht = hpool.tile([128, TOK_TILE], f32r, tag=f"h{j}")
                nc.vector.tensor_scalar(ht[:], ph[:], b1_sb[:, j:j + 1], 0.0,
                                        op0=ADD, op1=MAX)
                h_sb.append(ht)

            # layer 2 + gate: y^T[o,:] = (sum_j W2[j,o]^T h^T[j,:] + b2[o]) * g
            for o in range(8):
                py = pyp.tile([128, TOK_TILE], f32, tag="py")
                for j in range(8):
                    nc.tensor.matmul(py[:],
                                     w2_sb[j][:, o * 128:(o + 1) * 128],
                                     h_sb[j][:],
                                     start=(j == 0), stop=(j == 7))
                yt = ypool.tile([128, TOK_TILE], f32, tag=f"y{o}")
                nc.vector.scalar_tensor_tensor(yt[:], py[:], b2_sb[:, o:o + 1],
                                               g_bc[:], op0=ADD, op1=MULT)
                nc.sync.dma_start(yT_d[o * 128:(o + 1) * 128, tsl], yt[:])

    nc.compile()
    return nc


def _route(x, Wg, bg):
    """Host gating: fp32 softmax + top-2, matching jax.lax.top_k semantics."""
    logits = x @ Wg + bg
    m = logits.max(axis=1, keepdims=True)
    e = np.exp(logits - m)
    gates = e / e.sum(axis=1, keepdims=True)
    # stable argsort on negated values = ties broken by lower index (jax)
    order = np.argsort(-gates, axis=1, kind="stable")[:, :TOP_K]
    return gates, order


def kernel(x, W1, b1, W2, b2, Wg, bg):
    from concourse import bass_utils

    x = np.ascontiguousarray(np.asarray(x, dtype=np.float32))
    W1 = np.asarray(W1, dtype=np.float32)
    b1 = np.asarray(b1, dtype=np.float32)
    W2 = np.asarray(W2, dtype=np.float32)
    b2 = np.asarray(b2, dtype=np.float32)
    Wg = np.asarray(Wg, dtype=np.float32)
    bg = np.asarray(bg, dtype=np.float32)
    n = x.shape[0]

    gates, order = _route(x, Wg, bg)

    tok_lists = [np.where((order == e).any(axis=1))[0] for e in range(NUM_EXPERTS)]
    max_load = max(len(t) for t in tok_lists)
    C = -(-max_load // TOK_TILE) * TOK_TILE

    if C not in _prog_cache:
        _prog_cache[C] = _build_program(C)
    nc = _prog_cache[C]

    in_maps = []
    for e in range(NUM_EXPERTS):
        toks = tok_lists[e]
        ne = len(toks)
        xT_e = np.zeros((D, C), dtype=np.float32)
        xT_e[:, :ne] = x[toks].T
        g_e = np.zeros((1, C), dtype=np.float32)
        g_e[0, :ne] = gates[toks, e]
        in_maps.append({
            "xT": xT_e,
            "w1": np.ascontiguousarray(W1[e]),
            "w2": np.ascontiguousarray(W2[e]),
            "b1": np.ascontiguousarray(b1[e].reshape(8, 128, 1)),
            "b2": np.ascontiguousarray(b2[e].reshape(8, 128, 1)),
            "g": g_e,
            "ones": np.ones((1, 128), dtype=np.float32),
        })

    res = bass_utils.run_bass_kernel_spmd(nc, in_maps, list(range(NUM_EXPERTS)))
    yT_all = np.stack([res.results[e]["yT"] for e in range(NUM_EXPERTS)])

    # scatter-add the two expert contributions per token (already gated)
    slot = np.zeros((NUM_EXPERTS, n), dtype=np.int64)
    for e in range(NUM_EXPERTS):
        slot[e, tok_lists[e]] = np.arange(len(tok_lists[e]))
    rows = np.arange(n)
    out = yT_all[order[:, 0], :, slot[order[:, 0], rows]].astype(np.float32)
    for k in range(1, TOP_K):
        out += yT_all[order[:, k], :, slot[order[:, k], rows]]
    return out


# revision 13
# speedup vs baseline: 256.0970x; 256.0970x over previous
"""Trainium2 Bass kernel for an 8-expert top-2 MoE layer.

Strategy (expert-parallel, per the sharding hint "all-to-all tokens by
top-k assignment"): the host computes the (tiny) gating matmul + softmax
+ top-2 routing, gathers each expert's assigned tokens, and ships one
expert per NeuronCore. Each core runs the heavy 2-layer MLP for its
expert over its assigned tokens (f32r matmuls on the PE array), applies
the gate weights on-device, and the host scatter-adds the two expert
contributions per token back together.

The MLP math runs fully transposed (tokens on the free dim) so that
 - W1/W2 slices feed the PE as stationary operands with no transposes,
 - the b1 bias + relu and (y + b2) * gate evictions are single fused
   DVE ops with per-partition scalars,
 - the per-token gate row is broadcast across partitions with one K=1
   matmul (ones[1,128]^T @ g[1,N] -> G[128,N]).

DMA-dispatch overhead (~0.6us per dma_start, serialized on the DGE
queue) is first-order here, so transfers are batched: weights are
shipped as eight j-strip (o-strip) tensors host-packed to [128, 8, 128]
so each strip is one DMA and gates exactly one accumulation group, x
arrives as one DMA per token tile (host-packed [128, 8, C]), and the
biases land in one DMA each. The first token tile is 256 wide (f32r
matmul keeps full rate at free dim >= 256) to cut the startup ramp.
"""

import numpy as np

NUM_EXPERTS = 8
TOP_K = 2
D = 1024

_prog_cache = {}


def _plan_tiles(max_load):
    """Token-tile sizes covering max_load: 512s with a final 256 when it fits."""
    n256 = -(-max_load // 256)
    tiles = [512] * (n256 // 2)
    if n256 % 2 == 1:
        tiles.append(256)
    if not tiles:
        tiles = [256]
    return sum(tiles), tiles


def _build_program(tile_plan):
    """Build the per-core Bass program: one expert's MLP over C tokens."""
    from contextlib import ExitStack

    import concourse.tile as tile
    from concourse import bacc, mybir

    f32 = mybir.dt.float32
    f32r = mybir.dt.float32r
    ADD = mybir.AluOpType.add
    MAX = mybir.AluOpType.max
    MULT = mybir.AluOpType.mult

    C, tok_tiles = tile_plan

    nc = bacc.Bacc("TRN2", target_bir_lowering=False, debug=False,
                   num_devices=NUM_EXPERTS)

    # host-packed layouts (see _make_in_maps):
    #   xT:  [128, 8, C]      xT[p, d, c] = x_gathered[c, d*128+p]
    #   w1:  [8, 128, 8, 128] w1[j, p, d, r] = W1[d*128+p, j*128+r]
    #   w2:  [8, 128, 8, 128] w2[o, p, j, r] = W2[j*128+p, o*128+r]
    #   b1:  [128, 8]         b1[p, j] = b1[j*128+p]   (b2 same)
    #   yT:  [128, 8, C]      yT[p, o, c] = y[c, o*128+p] * gate[c]
    xT_d = nc.dram_tensor("xT", [128, 8, C], f32r, kind="ExternalInput").ap()
    w1_d = nc.dram_tensor("w1", [8, 128, 8, 128], f32r, kind="ExternalInput").ap()
    w2_d = nc.dram_tensor("w2", [8, 128, 8, 128], f32r, kind="ExternalInput").ap()
    b1_d = nc.dram_tensor("b1", [128, 8], f32, kind="ExternalInput").ap()
    b2_d = nc.dram_tensor("b2", [128, 8], f32, kind="ExternalInput").ap()
    g_d = nc.dram_tensor("g", [1, C], f32r, kind="ExternalInput").ap()
    ones_d = nc.dram_tensor("ones", [1, 128], f32r, kind="ExternalInput").ap()
    yT_d = nc.dram_tensor("yT", [128, 8, C], f32, kind="ExternalOutput").ap()

    with tile.TileContext(nc) as tc, ExitStack() as ctx:
        wpool = ctx.enter_context(tc.tile_pool(name="w", bufs=1))
        cpool = ctx.enter_context(tc.tile_pool(name="const", bufs=1))
        xpool = ctx.enter_context(tc.tile_pool(name="x", bufs=2))
        hpool = ctx.enter_context(tc.tile_pool(name="h", bufs=2))
        ypool = ctx.enter_context(tc.tile_pool(name="y", bufs=2))
        gpool = ctx.enter_context(tc.tile_pool(name="g", bufs=2))
        php = ctx.enter_context(tc.tile_pool(name="ph", bufs=3, space="PSUM"))
        pyp = ctx.enter_context(tc.tile_pool(name="py", bufs=3, space="PSUM"))
        pgp = ctx.enter_context(tc.tile_pool(name="pg", bufs=2, space="PSUM"))

        # tiny constants first so they never sit behind bulk DMA
        b1_sb = cpool.tile([128, 8], f32, tag="b1")
        nc.sync.dma_start(b1_sb[:], b1_d[:])
        b2_sb = cpool.tile([128, 8], f32, tag="b2")
        nc.sync.dma_start(b2_sb[:], b2_d[:])
        g_sb = cpool.tile([1, C], f32r, tag="g")
        nc.sync.dma_start(g_sb[:], g_d[:])
        ones_sb = cpool.tile([1, 128], f32r, tag="ones")
        nc.sync.dma_start(ones_sb[:], ones_d[:])

        # DMA emission in consumption order: w1 strip 0, then the first
        # token tile of x^T per d-block (the j=0 group's d-MMs start as each
        # block lands), then the remaining w1 strips (one gates each j-group)
        TT0 = tok_tiles[0]
        w1_sb = [None] * 8
        w1_first = wpool.tile([128, 8 * 128], f32r, tag="w1_0")
        nc.sync.dma_start(w1_first[:], w1_d[0])
        w1_sb[0] = w1_first
        x_sb0 = []
        for d in range(8):
            xt = xpool.tile([128, TT0], f32r, tag=f"x0_{d}")
            nc.sync.dma_start(xt[:], xT_d[:, d, 0:TT0])
            x_sb0.append(xt)
        for j in range(1, 8):
            w1_strip = wpool.tile([128, 8 * 128], f32r, tag=f"w1_{j}")
            nc.sync.dma_start(w1_strip[:], w1_d[j])
            w1_sb[j] = w1_strip

        # w2 o-strips next: strip o gates tile 0's layer-2 o-group, which
        # starts right after layer 1 (~the w1 stream), so these must not
        # queue behind the second x tile; the second x tile (needed only
        # when tile 0 fully finishes) slots in before the last strip
        x_tiles = [None] * len(tok_tiles)
        x_tiles[0] = x_sb0
        w2_sb = [None] * 8
        for o in range(8):
            if o == 7 and len(tok_tiles) > 1:
                x_next = xpool.tile([128, 8 * tok_tiles[1]], f32r, tag="x")
                nc.sync.dma_start(x_next[:],
                                  xT_d[:, :, TT0:TT0 + tok_tiles[1]])
                x_tiles[1] = x_next
            w2_strip = wpool.tile([128, 8 * 128], f32r, tag=f"w2_{o}")
            nc.sync.dma_start(w2_strip[:], w2_d[o])
            w2_sb[o] = w2_strip

        tile_pos = np.cumsum([0] + tok_tiles).tolist()
        pos = 0
        for t, TT in enumerate(tok_tiles):
            tsl = slice(pos, pos + TT)

            # prefetch x for tile t+1 (tiles 0 and 1 already issued)
            nt = t + 1
            if nt < len(tok_tiles) and x_tiles[nt] is None:
                x_pref = xpool.tile([128, 8 * tok_tiles[nt]], f32r, tag="x")
                nc.sync.dma_start(
                    x_pref[:],
                    xT_d[:, :, tile_pos[nt]:tile_pos[nt] + tok_tiles[nt]])
                x_tiles[nt] = x_pref

            x_sb = x_tiles[t]

            def xs(d):
                if t == 0:
                    return x_sb[d][:]
                return x_sb[:, d * TT:(d + 1) * TT]

            # broadcast gate row across partitions: G[p, n] = g[n]
            g_ps = pgp.tile([128, TT], f32, tag="gps")
            nc.tensor.matmul(g_ps[:], ones_sb[:], g_sb[:, tsl],
                             start=True, stop=True)
            g_bc = gpool.tile([128, TT], f32, tag="gbc")
            nc.vector.tensor_copy(g_bc[:], g_ps[:])

            # layer 1: h^T[j,:] = relu(sum_d W1[d,j]^T x^T[d,:] + b1[j])
            h_sb = []
            for j in range(8):
                ph = php.tile([128, TT], f32, tag="ph")
                for d in range(8):
                    nc.tensor.matmul(ph[:],
                                     w1_sb[j][:, d * 128:(d + 1) * 128],
                                     xs(d),
                                     start=(d == 0), stop=(d == 7))
                ht = hpool.tile([128, TT], f32r, tag=f"h{j}")
                nc.vector.tensor_scalar(ht[:], ph[:], b1_sb[:, j:j + 1], 0.0,
                                        op0=ADD, op1=MAX)
                h_sb.append(ht)

            # layer 2 + gate: y^T[o,:] = (sum_j W2[j,o]^T h^T[j,:] + b2[o]) * g
            for o in range(8):
                py = pyp.tile([128, TT], f32, tag="py")
                for j in range(8):
                    nc.tensor.matmul(py[:],
                                     w2_sb[o][:, j * 128:(j + 1) * 128],
                                     h_sb[j][:],
                                     start=(j == 0), stop=(j == 7))
                yt = ypool.tile([128, TT], f32, tag=f"y{o}")
                nc.vector.scalar_tensor_tensor(yt[:], py[:], b2_sb[:, o:o + 1],
                                               g_bc[:], op0=ADD, op1=MULT)
                nc.sync.dma_start(yT_d[:, o, tsl], yt[:])

            pos += TT

    nc.compile()
    return nc


def _route(x, Wg, bg):
    """Host gating: fp32 softmax + top-2, matching jax.lax.top_k semantics."""
    logits = x @ Wg + bg
    m = logits.max(axis=1, keepdims=True)
    e = np.exp(logits - m)
    gates = e / e.sum(axis=1, keepdims=True)
    # stable argsort on negated values = ties broken by lower index (jax)
    order = np.argsort(-gates, axis=1, kind="stable")[:, :TOP_K]
    return gates, order


def _pack_w(W):
    """[1024, 1024] -> [8, 128, 8, 128]: strip s, part p, rowtile d, col r."""
    # out[s, p, d, r] = W[d*128+p, s*128+r]
    return np.ascontiguousarray(
        W.reshape(8, 128, 8, 128).transpose(2, 1, 0, 3))


def _make_in_maps(x, W1, b1, W2, b2, gates, order, tok_lists, C):
    in_maps = []
    for e in range(NUM_EXPERTS):
        toks = tok_lists[e]
        ne = len(toks)
        xT_e = np.zeros((128, 8, C), dtype=np.float32)
        # xT_e[p, d, :ne] = x[toks, d*128+p].T
        xT_e[:, :, :ne] = x[toks].T.reshape(8, 128, ne).transpose(1, 0, 2)
        g_e = np.zeros((1, C), dtype=np.float32)
        g_e[0, :ne] = gates[toks, e]
        in_maps.append({
            "xT": xT_e,
            "w1": _pack_w(W1[e]),
            "w2": _pack_w(W2[e]),
            "b1": np.ascontiguousarray(b1[e].reshape(8, 128).T),
            "b2": np.ascontiguousarray(b2[e].reshape(8, 128).T),
            "g": g_e,
            "ones": np.ones((1, 128), dtype=np.float32),
        })
    return in_maps


def kernel(x, W1, b1, W2, b2, Wg, bg):
    from concourse import bass_utils

    x = np.ascontiguousarray(np.asarray(x, dtype=np.float32))
    W1 = np.asarray(W1, dtype=np.float32)
    b1 = np.asarray(b1, dtype=np.float32)
    W2 = np.asarray(W2, dtype=np.float32)
    b2 = np.asarray(b2, dtype=np.float32)
    Wg = np.asarray(Wg, dtype=np.float32)
    bg = np.asarray(bg, dtype=np.float32)
    n = x.shape[0]

    gates, order = _route(x, Wg, bg)
    tok_lists = [np.where((order == e).any(axis=1))[0] for e in range(NUM_EXPERTS)]
    max_load = max(len(t) for t in tok_lists)
    C, tok_tiles = _plan_tiles(max_load)

    key = (C, tuple(tok_tiles))
    if key not in _prog_cache:
        _prog_cache[key] = _build_program((C, tok_tiles))
    nc = _prog_cache[key]

    in_maps = _make_in_maps(x, W1, b1, W2, b2, gates, order, tok_lists, C)
    res = bass_utils.run_bass_kernel_spmd(nc, in_maps, list(range(NUM_EXPERTS)))
    # yT result: [128, 8, C] -> y_e[c, o*128+p] = yT[p, o, c]
    yT_all = np.stack([res.results[e]["yT"] for e in range(NUM_EXPERTS)])

    # scatter-add the two expert contributions per token (already gated)
    slot = np.zeros((NUM_EXPERTS, n), dtype=np.int64)
    for e in range(NUM_EXPERTS):
        slot[e, tok_lists[e]] = np.arange(len(tok_lists[e]))
    rows = np.arange(n)
    # gather columns: result [n, 128, 8] -> reshape to [n, 1024]
    out = np.zeros((n, D), dtype=np.float32)
    for k in range(TOP_K):
        ek = order[:, k]
        picked = yT_all[ek, :, :, slot[ek, rows]]   # [n, 128, 8]
        out += picked.transpose(0, 2, 1).reshape(n, D)
    return out


# revision 24
# speedup vs baseline: 258.4021x; 1.0090x over previous
"""Trainium2 Bass kernel for an 8-expert top-2 MoE layer.

Strategy (expert-parallel, per the sharding hint "all-to-all tokens by
top-k assignment"): the host computes the (tiny) gating matmul + softmax
+ top-2 routing, gathers each expert's assigned tokens, and ships one
expert per NeuronCore. Each core runs the heavy 2-layer MLP for its
expert over its assigned tokens (f32r matmuls on the PE array), applies
the gate weights on-device, and the host scatter-adds the two expert
contributions per token back together.

The MLP math runs fully transposed (tokens on the free dim) so that
 - W1/W2 slices feed the PE as stationary operands with no transposes,
 - the b1 bias + relu and (y + b2) * gate evictions are single fused
   DVE ops with per-partition scalars,
 - the per-token gate row is broadcast across partitions with one K=1
   matmul (ones[1,128]^T @ g[1,N] -> G[128,N]).

DMA-dispatch overhead (~0.6us per dma_start, serialized on the DGE
queue) is first-order here, so transfers are batched: weights are
shipped as eight j-strip (o-strip) tensors host-packed to [128, 8, 128]
so each strip is one DMA and gates exactly one accumulation group, x
arrives as one DMA per token tile (host-packed [128, 8, C]), and the
biases land in one DMA each. The first token tile is 256 wide (f32r
matmul keeps full rate at free dim >= 256) to cut the startup ramp.
"""

import numpy as np

NUM_EXPERTS = 8
TOP_K = 2
D = 1024

_prog_cache = {}


def _plan_tiles(max_load):
    """Token-tile sizes covering max_load: 512s with a final 256 when it fits."""
    n256 = -(-max_load // 256)
    tiles = [512] * (n256 // 2)
    if n256 % 2 == 1:
        tiles.append(256)
    if not tiles:
        tiles = [256]
    return sum(tiles), tiles


def _build_program(tile_plan):
    """Build the per-core Bass program: one expert's MLP over C tokens."""
    from contextlib import ExitStack

    import concourse.tile as tile
    from concourse import bacc, mybir

    f32 = mybir.dt.float32
    f32r = mybir.dt.float32r
    ADD = mybir.AluOpType.add
    MAX = mybir.AluOpType.max
    MULT = mybir.AluOpType.mult

    C, tok_tiles = tile_plan

    nc = bacc.Bacc("TRN2", target_bir_lowering=False, debug=False,
                   num_devices=NUM_EXPERTS)

    # host-packed layouts (see _make_in_maps):
    #   xT:  [128, 8, C]      xT[p, d, c] = x_gathered[c, d*128+p]
    #   w1:  [8, 128, 8, 128] w1[j, p, d, r] = W1[d*128+p, j*128+r]
    #   w2:  [8, 128, 8, 128] w2[o, p, j, r] = W2[j*128+p, o*128+r]
    #   b1:  [128, 8]         b1[p, j] = b1[j*128+p]   (b2 same)
    #   yT:  [128, 8, C]      yT[p, o, c] = y[c, o*128+p] * gate[c]
    xT_d = nc.dram_tensor("xT", [128, 8, C], f32r, kind="ExternalInput").ap()
    w1_d = nc.dram_tensor("w1", [8, 128, 8, 128], f32r, kind="ExternalInput").ap()
    w2_d = nc.dram_tensor("w2", [8, 128, 8, 128], f32r, kind="ExternalInput").ap()
    bb_d = nc.dram_tensor("bb", [128, 16], f32, kind="ExternalInput").ap()
    go_d = nc.dram_tensor("go", [1, C + 128], f32r, kind="ExternalInput").ap()
    yT_d = nc.dram_tensor("yT", [128, 8, C], f32, kind="ExternalOutput").ap()

    with tile.TileContext(nc) as tc, ExitStack() as ctx:
        wpool = ctx.enter_context(tc.tile_pool(name="w", bufs=1))
        cpool = ctx.enter_context(tc.tile_pool(name="const", bufs=1))
        xpool = ctx.enter_context(tc.tile_pool(name="x", bufs=2))
        hpool = ctx.enter_context(tc.tile_pool(name="h", bufs=2))
        ypool = ctx.enter_context(tc.tile_pool(name="y", bufs=2))
        gpool = ctx.enter_context(tc.tile_pool(name="g", bufs=2))
        php = ctx.enter_context(tc.tile_pool(name="ph", bufs=3, space="PSUM"))
        pyp = ctx.enter_context(tc.tile_pool(name="py", bufs=3, space="PSUM"))
        pgp = ctx.enter_context(tc.tile_pool(name="pg", bufs=2, space="PSUM"))

        # tiny constants on the ACT DGE queue (parallel with the weight
        # stream on the SP queue), merged into single transfers:
        # bb = [b1 | b2] per-partition, go = [gate row | ones row]
        bb_sb = cpool.tile([128, 16], f32, tag="bb")
        nc.sync.dma_start(bb_sb[:], bb_d[:])
        b1_sb = bb_sb[:, 0:8]
        b2_sb = bb_sb[:, 8:16]
        go_sb = cpool.tile([1, C + 128], f32r, tag="go")
        nc.sync.dma_start(go_sb[:], go_d[:])
        g_sb = go_sb[:, 0:C]
        ones_sb = go_sb[:, C:C + 128]

        # PE warm-up in the shadow of the initial DMA ramp: ~4us of dummy
        # K=1 matmuls (gated only on the tiny g/ones transfers) keep the
        # HAM activity monitor busy so the real matmuls run at 2.4 GHz
        warm = pgp.tile([128, min(C, 512)], f32, tag="gps")
        for _ in range(6):
            nc.tensor.matmul(warm[:], ones_sb[:], g_sb[:, 0:min(C, 512)],
                             start=True, stop=True)

        # DMA emission in consumption order: w1 strip 0, then the first
        # token tile of x^T per d-block (the j=0 group's d-MMs start as each
        # block lands), then the remaining w1 strips (one gates each j-group)
        TT0 = tok_tiles[0]
        w1_sb = [None] * 8
        w1_first = wpool.tile([128, 8 * 128], f32r, tag="w1_0")
        nc.sync.dma_start(w1_first[:], w1_d[0])
        w1_sb[0] = w1_first
        x_sb0 = xpool.tile([128, 8 * TT0], f32r, tag="x")
        nc.sync.dma_start(x_sb0[:], xT_d[:, :, 0:TT0])
        for j in range(1, 8):
            w1_strip = wpool.tile([128, 8 * 128], f32r, tag=f"w1_{j}")
            nc.sync.dma_start(w1_strip[:], w1_d[j])
            w1_sb[j] = w1_strip

        # w2 o-strips next: strip o gates tile 0's layer-2 o-group, which
        # starts right after layer 1 (~the w1 stream), so these must not
        # queue behind the second x tile; the second x tile (needed only
        # when tile 0 fully finishes) slots in before the last strip
        x_tiles = [None] * len(tok_tiles)
        x_tiles[0] = x_sb0
        w2_sb = [None] * 8
        for o in range(8):
            if o == 7 and len(tok_tiles) > 1:
                x_next = xpool.tile([128, 8 * tok_tiles[1]], f32r, tag="x")
                nc.sync.dma_start(x_next[:],
                                    xT_d[:, :, TT0:TT0 + tok_tiles[1]])
                x_tiles[1] = x_next
            w2_strip = wpool.tile([128, 8 * 128], f32r, tag=f"w2_{o}")
            nc.sync.dma_start(w2_strip[:], w2_d[o])
            w2_sb[o] = w2_strip

        tile_pos = np.cumsum([0] + tok_tiles).tolist()
        pos = 0
        for t, TT in enumerate(tok_tiles):
            tsl = slice(pos, pos + TT)

            # prefetch x for tile t+1 (tiles 0 and 1 already issued)
            nt = t + 1
            if nt < len(tok_tiles) and x_tiles[nt] is None:
                x_pref = xpool.tile([128, 8 * tok_tiles[nt]], f32r, tag="x")
                nc.sync.dma_start(
                    x_pref[:],
                    xT_d[:, :, tile_pos[nt]:tile_pos[nt] + tok_tiles[nt]])
                x_tiles[nt] = x_pref

            x_sb = x_tiles[t]

            def xs(d):
                return x_sb[:, d * TT:(d + 1) * TT]

            # broadcast gate row across partitions: G[p, n] = g[n]
            g_ps = pgp.tile([128, TT], f32, tag="gps")
            nc.tensor.matmul(g_ps[:], ones_sb[:], g_sb[:, tsl],
                             start=True, stop=True)
            g_bc = gpool.tile([128, TT], f32, tag="gbc")
            nc.vector.tensor_copy(g_bc[:], g_ps[:])

            # layer 1: h^T[j,:] = relu(sum_d W1[d,j]^T x^T[d,:] + b1[j])
            h_sb = []
            for j in range(8):
                ph = php.tile([128, TT], f32, tag="ph")
                for d in range(8):
                    nc.tensor.matmul(ph[:],
                                     w1_sb[j][:, d * 128:(d + 1) * 128],
                                     xs(d),
                                     start=(d == 0), stop=(d == 7))
                ht = hpool.tile([128, TT], f32r, tag=f"h{j}")
                nc.vector.tensor_scalar(ht[:], ph[:], b1_sb[:, j:j + 1], 0.0,
                                        op0=ADD, op1=MAX)
                h_sb.append(ht)

            # layer 2 + gate: y^T[o,:] = (sum_j W2[j,o]^T h^T[j,:] + b2[o]) * g
            for o in range(8):
                py = pyp.tile([128, TT], f32, tag="py")
                for j in range(8):
                    nc.tensor.matmul(py[:],
                                     w2_sb[o][:, j * 128:(j + 1) * 128],
                                     h_sb[j][:],
                                     start=(j == 0), stop=(j == 7))
                yt = ypool.tile([128, TT], f32, tag=f"y{o}")
                nc.vector.scalar_tensor_tensor(yt[:], py[:], b2_sb[:, o:o + 1],
                                               g_bc[:], op0=ADD, op1=MULT)
                nc.sync.dma_start(yT_d[:, o, tsl], yt[:])

            pos += TT

    nc.compile()
    return nc


def _route(x, Wg, bg):
    """Host gating: fp32 softmax + top-2, matching jax.lax.top_k semantics."""
    logits = x @ Wg + bg
    m = logits.max(axis=1, keepdims=True)
    e = np.exp(logits - m)
    gates = e / e.sum(axis=1, keepdims=True)
    # stable argsort on negated values = ties broken by lower index (jax)
    order = np.argsort(-gates, axis=1, kind="stable")[:, :TOP_K]
    return gates, order


def _pack_w(W):
    """[1024, 1024] -> [8, 128, 8, 128]: strip s, part p, rowtile d, col r."""
    # out[s, p, d, r] = W[d*128+p, s*128+r]
    return np.ascontiguousarray(
        W.reshape(8, 128, 8, 128).transpose(2, 1, 0, 3))


def _make_in_maps(x, W1, b1, W2, b2, gates, order, tok_lists, C):
    in_maps = []
    for e in range(NUM_EXPERTS):
        toks = tok_lists[e]
        ne = len(toks)
        xT_e = np.zeros((128, 8, C), dtype=np.float32)
        # xT_e[p, d, :ne] = x[toks, d*128+p].T
        xT_e[:, :, :ne] = x[toks].T.reshape(8, 128, ne).transpose(1, 0, 2)
        g_e = np.zeros((1, C), dtype=np.float32)
        g_e[0, :ne] = gates[toks, e]
        in_maps.append({
            "xT": xT_e,
            "w1": _pack_w(W1[e]),
            "w2": _pack_w(W2[e]),
            "bb": np.ascontiguousarray(np.concatenate(
                [b1[e].reshape(8, 128).T, b2[e].reshape(8, 128).T], axis=1)),
            "go": np.concatenate(
                [g_e, np.ones((1, 128), dtype=np.float32)], axis=1),
        })
    return in_maps


def kernel(x, W1, b1, W2, b2, Wg, bg):
    from concourse import bass_utils

    x = np.ascontiguousarray(np.asarray(x, dtype=np.float32))
    W1 = np.asarray(W1, dtype=np.float32)
    b1 = np.asarray(b1, dtype=np.float32)
    W2 = np.asarray(W2, dtype=np.float32)
    b2 = np.asarray(b2, dtype=np.float32)
    Wg = np.asarray(Wg, dtype=np.float32)
    bg = np.asarray(bg, dtype=np.float32)
    n = x.shape[0]

    gates, order = _route(x, Wg, bg)
    tok_lists = [np.where((order == e).any(axis=1))[0] for e in range(NUM_EXPERTS)]
    max_load = max(len(t) for t in tok_lists)
    C, tok_tiles = _plan_tiles(max_load)

    key = (C, tuple(tok_tiles))
    if key not in _prog_cache:
        _prog_cache[key] = _build_program((C, tok_tiles))
    nc = _prog_cache[key]

    in_maps = _make_in_maps(x, W1, b1, W2, b2, gates, order, tok_lists, C)
    res = bass_utils.run_bass_kernel_spmd(nc, in_maps, list(range(NUM_EXPERTS)))
    # yT result: [128, 8, C] -> y_e[c, o*128+p] = yT[p, o, c]
    yT_all = np.stack([res.results[e]["yT"] for e in range(NUM_EXPERTS)])

    # scatter-add the two expert contributions per token (already gated)
    slot = np.zeros((NUM_EXPERTS, n), dtype=np.int64)
    for e in range(NUM_EXPERTS):
        slot[e, tok_lists[e]] = np.arange(len(tok_lists[e]))
    rows = np.arange(n)
    # gather columns: result [n, 128, 8] -> reshape to [n, 1024]
    out = np.zeros((n, D), dtype=np.float32)
    for k in range(TOP_K):
        ek = order[:, k]
        picked = yT_all[ek, :, :, slot[ek, rows]]   # [n, 128, 8]
        out += picked.transpose(0, 2, 1).reshape(n, D)
    return out
